# revision 35
# baseline (speedup 1.0000x reference)
"""Self-contained Trainium2 Bass kernel for the ARMA GNN problem
(nn_ARMA_49297634623854).

kernel(**inputs) takes the FULL unsharded inputs (x, edge_index, edge_attr,
batch, Wi1, Wr1, b1, Wi2, Wr2, b2, mW1, mb1, mW2, mb2) as numpy arrays,
shards node-contiguously across 8 NeuronCores, runs an SPMD Bass kernel
(batched dma_gather edge gather + one-hot-matmul scatter fused in PSUM +
AllGather/AllReduce collectives), and returns the full [512, 2] float32
output.

Design notes:
- gcn_norm (deg/dinv) is computed on the host and baked into per-edge
  weights, so the device never computes degrees or rescales messages.
- Edge messages m = h @ Wi are AllGathered (bf16), then gathered per-edge
  with batched dma_gather (int16 indices -> 4 source-range groups of 25000
  rows each).
- Edges are grouped by (source-range group, 128-wide dest window); for each
  window all scatter matmuls plus the Wr-path matmul accumulate into one
  PSUM tile, and a single scalar-engine activation (relu + bias) writes the
  new hT window. No aggregation buffer, no vector adds.
"""

# ======================= walrus wait-splitting patches =======================
import concourse.mybir as mybir
import concourse.tile as tile
from concourse.vector_clock import ScopedClock, VectorClock

_nop_counter = [0]


def _make_wait_nop(engine, wait):
    _nop_counter[0] += 1
    return mybir.InstNoOp(
        name=f"SplitWait-{_nop_counter[0]}",
        engine=engine,
        ins=[],
        outs=[],
        sync_info=mybir.SyncInfo(on_wait=[wait], on_update=[]),
        bass_nofuse=True,
    )


def _split_multi_waits(insts):
    out = []
    n_split = 0
    for inst in insts:
        si = inst.sync_info
        if si is not None and len(si.on_wait) > 1:
            waits = list(si.on_wait)
            for w in waits[:-1]:
                out.append(_make_wait_nop(inst.engine, w))
            inst.sync_info = mybir.SyncInfo(
                on_wait=[waits[-1]], on_update=list(si.on_update)
            )
            n_split += 1
        out.append(inst)
    return out, n_split


_orig_lower = tile.TileContext._lower_ordered_insts


def _patched_lower(self, postordered_blocks):
    total = 0
    for bbname in list(postordered_blocks.keys()):
        newlist, n = _split_multi_waits(postordered_blocks[bbname])
        postordered_blocks[bbname] = newlist
        total += n
    return _orig_lower(self, postordered_blocks)


def _patched_drain_and_barrier(self, tick_clock, wait_clock):
    gc = tick_clock.global_clock
    nprocs = len(gc)
    for p in range(nprocs):
        t = gc[p]
        if t <= 0:
            continue
        vec = [0] * nprocs
        vec[p] = t
        nop_inst = self.nc.sync.nop(nofuse=True)
        wait_clock.add_sem_waits(nop_inst.ins, ScopedClock({None: VectorClock(vec)}))
    self.nc.sync.drain()
    self.nc.all_engine_barrier()
    assert self.sems is not None
    popped = self.nc._tile_sem_poison_stack.pop()
    assert popped is self._sem_poison
    self.nc.clear_and_free_semaphores(list(self.sems.allocated().values()))
    self.nc.all_engine_barrier()


def install():
    tile.TileContext._lower_ordered_insts = _patched_lower
    tile.TileContext._drain_and_barrier = _patched_drain_and_barrier


# ======================= SPMD runner =======================
import time

import jax
import numpy as np
from jax.sharding import Mesh, NamedSharding, PartitionSpec
from jax.experimental.shard_map import shard_map

import concourse.bass as bass
import concourse.mybir as mybir
from concourse import bass2jax
from concourse.bass2jax import (
    _bass_exec_p,
    fast_dispatch_compile,
    install_neuronx_cc_hook,
    partition_id_tensor,
)


class SpmdKernel:
    def __init__(self, nc: bass.Bass, n_cores: int = 8):
        install_neuronx_cc_hook()
        self.nc = nc
        self.n_cores = n_cores
        in_names: list[str] = []
        out_names: list[str] = []
        out_avals: list[jax.core.ShapedArray] = []
        partition_name = (
            nc.partition_id_tensor.name if nc.partition_id_tensor else None
        )
        for alloc in nc.m.functions[0].allocations:
            if not isinstance(alloc, mybir.MemoryLocationSet):
                continue
            name = alloc.memorylocations[0].name
            if alloc.kind == "ExternalInput":
                if name != partition_name:
                    in_names.append(name)
            elif alloc.kind == "ExternalOutput":
                shape = tuple(alloc.tensor_shape)
                dtype = mybir.dt.np(alloc.dtype)
                out_names.append(name)
                out_avals.append(jax.core.ShapedArray(shape, dtype))
        self.n_params = len(in_names)
        self.out_names = out_names
        self.out_avals = out_avals
        self.in_names = in_names[:]
        all_in_names = in_names + out_names
        if partition_name is not None:
            all_in_names.append(partition_name)

        def _body(*args):
            operands = list(args)
            if partition_name is not None:
                operands.append(partition_id_tensor())
            outs = _bass_exec_p.bind(
                *operands,
                out_avals=tuple(out_avals),
                in_names=tuple(all_in_names),
                out_names=tuple(out_names),
                lowering_input_output_aliases=(),
                sim_require_finite=True,
                sim_require_nnan=True,
                nc=nc,
            )
            return tuple(outs)

        devices = jax.devices()[: n_cores]
        assert len(devices) == n_cores
        self.mesh = Mesh(np.asarray(devices), ("core",))
        n_out = len(out_names)
        in_specs = (PartitionSpec("core"),) * (self.n_params + n_out)
        out_specs = (PartitionSpec("core"),) * n_out
        self._sharded = shard_map(
            _body,
            mesh=self.mesh,
            in_specs=in_specs,
            out_specs=out_specs,
            check_rep=False,
        )
        self.fn = jax.jit(self._sharded, keep_unused=True)
        self._compiled = None
        self.sharding = NamedSharding(self.mesh, PartitionSpec("core"))

    def compile_fast(self, concat_in, zeros):
        """AOT compile with fast dispatch (no effects)."""
        self._compiled = fast_dispatch_compile(
            lambda: jax.jit(self._sharded, keep_unused=True)
            .lower(*concat_in, *zeros)
            .compile()
        )
        return self._compiled

    def put_inputs(self, in_maps: list[dict[str, np.ndarray]]):
        """in_maps: one dict per core. Returns list of device arrays (concat
        along axis 0) in in_names order, plus zero output buffers."""
        concat_in = []
        for name in self.in_names:
            arrs = [np.asarray(in_maps[c][name]) for c in range(self.n_cores)]
            concat_in.append(
                jax.device_put(np.concatenate(arrs, axis=0), self.sharding)
            )
        zeros = []
        for av in self.out_avals:
            z = np.zeros((self.n_cores * av.shape[0], *av.shape[1:]), av.dtype)
            zeros.append(jax.device_put(z, self.sharding))
        return concat_in, zeros

    def __call__(self, concat_in, zeros):
        f = self._compiled or self.fn
        outs = f(*concat_in, *zeros)
        return outs

    def run_np(self, concat_in, zeros):
        f = self._compiled or self.fn
        outs = f(*concat_in, *zeros)
        res = []
        for c in range(self.n_cores):
            res.append(
                {
                    name: np.asarray(outs[i]).reshape(
                        self.n_cores, *self.out_avals[i].shape
                    )[c]
                    for i, name in enumerate(self.out_names)
                }
            )
        return res

    def time_it(self, concat_in, zeros, reps=20, warmup=3):
        f = self._compiled or self.fn
        for _ in range(warmup):
            jax.block_until_ready(f(*concat_in, *zeros))
        ts = []
        for _ in range(reps):
            t0 = time.perf_counter()
            jax.block_until_ready(f(*concat_in, *zeros))
            ts.append(time.perf_counter() - t0)
        return min(ts), sorted(ts)[len(ts) // 2]


# ======================= GNN kernel builder =======================
import concourse.bass as bass
import concourse.mybir as mybir
import concourse.tile as tile

F32 = mybir.dt.float32
BF16 = mybir.dt.bfloat16
I16 = mybir.dt.int16
AF = mybir.ActivationFunctionType
OP = mybir.AluOpType
P = 128
RANGE = 25000  # int16-safe dma_gather source range
NSB = 8        # gather superblocks per layer per group


def wrap128(arr):
    """[C*128,...] -> [128, C] layout: out[p, c] = arr[c*128+p]."""
    C = arr.shape[0] // 128
    return np.ascontiguousarray(arr.reshape(C, 128).T)


def to_bf16(a):
    import jax.numpy as jnp
    return np.asarray(jnp.asarray(np.asarray(a, np.float32), dtype=jnp.bfloat16))


def preprocess(edge_index, edge_attr, batch, n, ncores, g):
    """Host-side: gcn_norm, per-core edge->chunk tables, pooling tables.

    Chunks are 128 edge slots grouped by (source-range group, 512-wide dest
    quad). Chunk ordering: for sb: for group: for quad in sb: chunks -- so
    each (sb, group) is one contiguous dma_gather call span.
    """
    npc = n // ncores
    nw = (npc + 127) // 128          # 128-node chunks (pooling, m-pass)
    QW = 512                          # scatter quad width
    nq = (npc + QW - 1) // QW
    npc_pad = nq * QW                 # hT padded to whole quads
    ngroups = (n + RANGE - 1) // RANGE
    gblocks = (g + P - 1) // P

    row = np.asarray(edge_index[0], np.int64)
    col = np.asarray(edge_index[1], np.int64)
    w_attr = np.asarray(edge_attr, np.float32).reshape(-1)
    batch = np.asarray(batch, np.int64)

    deg = np.bincount(col, weights=w_attr.astype(np.float64), minlength=n)
    dinv = np.where(deg > 0, 1.0 / np.sqrt(np.maximum(deg, 1e-12)), 0.0)
    norm = (dinv[row] * w_attr * dinv[col]).astype(np.float32)

    per_core = []
    cnts = np.zeros((ncores, ngroups, nq), np.int64)
    for c in range(ncores):
        m = (col // npc) == c
        r_c = row[m]
        d_c = col[m] - c * npc
        n_c = norm[m]
        g_c = r_c // RANGE
        q_c = d_c // QW
        o = np.lexsort((r_c, q_c, g_c))
        r_c, d_c, n_c, g_c, q_c = r_c[o], d_c[o], n_c[o], g_c[o], q_c[o]
        per_core.append((r_c, d_c, n_c, g_c, q_c))
        for gi in range(ngroups):
            cnts[c, gi] = np.bincount(q_c[g_c == gi], minlength=nq)
    nch = ((cnts.max(axis=0) + P - 1) // P).astype(np.int64)  # [ngroups, nq]

    # quad -> superblock (balanced, monotone)
    qsb = (np.arange(nq) * NSB) // nq
    chunk_of = {}
    calls = []  # (sb, g, c0, c1)
    gq_span = {}  # (g, q) -> (call_idx, local0, k)
    C = 0
    for sb in range(NSB):
        qs = [q for q in range(nq) if qsb[q] == sb]
        for gi in range(ngroups):
            c0 = C
            ci_call = len(calls)
            for q in qs:
                chunk_of[(gi, q)] = C
                gq_span[(gi, q)] = (ci_call, C - c0, int(nch[gi, q]))
                C += int(nch[gi, q])
            calls.append((sb, gi, c0, C))
    kbmax = max(c1 - c0 for (_, _, c0, c1) in calls)
    kgqmax = int(nch.max())

    cores = []
    for c in range(ncores):
        r_c, d_c, n_c, g_c, q_c = per_core[c]
        idx16 = np.zeros(C * P, np.int16)
        colloc = np.zeros(C * P, np.float32)
        wvals = np.zeros(C * P, np.float32)
        key = g_c * nq + q_c
        uniq, starts = np.unique(key, return_index=True)
        ends = np.r_[starts[1:], len(key)]
        for u, s0, s1 in zip(uniq, starts, ends):
            gi, q = divmod(int(u), nq)
            base = chunk_of[(gi, q)] * P
            cnt = int(s1 - s0)
            sl = slice(base, base + cnt)
            idx16[sl] = (r_c[s0:s1] - gi * RANGE).astype(np.int16)
            colloc[sl] = (d_c[s0:s1] - q * QW).astype(np.float32)
            wvals[sl] = n_c[s0:s1]
        idx_t = np.tile(np.ascontiguousarray(idx16.reshape(C * 8, 16).T), (8, 1))
        cores.append(
            dict(
                idxs=np.ascontiguousarray(idx_t),
                colloc=wrap128(colloc),
                wvals=wrap128(wvals),
            )
        )

    # pooling: per-core local graph index (slab spans <= 128 graphs)
    for c in range(ncores):
        bl = batch[c * npc:(c + 1) * npc]
        g0, g1 = int(bl[0]), int(bl[-1])
        assert g1 - g0 + 1 <= P, f"core {c} graph span {g1 - g0 + 1} > 128"
        blloc = np.full(nw * P, -1.0, np.float32)
        blloc[:npc] = (bl - g0).astype(np.float32)
        cores[c]["batchloc"] = wrap128(blloc)  # [P, nw]
        cores[c]["growidx"] = (g0 + np.arange(P, dtype=np.int32)).reshape(P, 1)

    cnt = np.bincount(batch, minlength=g).astype(np.float64)
    cnt_inv = (1.0 / np.maximum(cnt, 1.0)).astype(np.float32)  # [g]
    cntinv_t = np.ascontiguousarray(cnt_inv.reshape(gblocks, P).T)  # [P, gblocks]

    return dict(
        C=C, calls=calls, gq_span=gq_span, kbmax=kbmax, kgqmax=kgqmax,
        nw=nw, nq=nq, QW=QW, npc=npc, npc_pad=npc_pad, ngroups=ngroups,
        gblocks=gblocks, cores=cores, cntinv=cntinv_t,
    )


def build_nc(pre, n, ncores, g, ncls):
    F = 128
    C = pre["C"]
    nw = pre["nw"]
    nq = pre["nq"]
    QW = pre["QW"]
    npc = pre["npc"]
    npc_pad = pre["npc_pad"]
    ngroups = pre["ngroups"]
    gblocks = pre["gblocks"]
    calls = pre["calls"]
    gq_span = pre["gq_span"]
    kbmax = pre["kbmax"]
    kgqmax = pre["kgqmax"]
    # descriptor-ring capacity check: ~255 descs/engine-ring with 32KB scratch
    assert kbmax * P // 16 + 1 <= 248, f"gather call too big: kbmax={kbmax}"

    nc = bass.Bass(dynamic_dma_scratch_size=32768)

    # ---------------- parameters ----------------
    xT = nc.declare_dram_parameter("xT", [P, npc_pad], BF16, isOutput=False)
    idxs_p = nc.declare_dram_parameter("idxs", [P, C * 8], I16, isOutput=False)
    colloc = nc.declare_dram_parameter("colloc", [P, C], F32, isOutput=False)
    wvals = nc.declare_dram_parameter("wvals", [P, C], F32, isOutput=False)
    batchloc = nc.declare_dram_parameter("batchloc", [P, nw], F32, isOutput=False)
    growidx = nc.declare_dram_parameter("growidx", [P, 1], mybir.dt.int32, isOutput=False)
    cntinv = nc.declare_dram_parameter("cntinv", [P, gblocks], F32, isOutput=False)
    iota_p = nc.declare_dram_parameter("iota512", [P, QW], F32, isOutput=False)
    ident_p = nc.declare_dram_parameter("ident128", [P, P], F32, isOutput=False)
    wi = [nc.declare_dram_parameter(f"wi{l}", [F, F], BF16, isOutput=False) for l in (1, 2)]
    wr = [nc.declare_dram_parameter(f"wr{l}", [F, F], BF16, isOutput=False) for l in (1, 2)]
    bb = [nc.declare_dram_parameter(f"b{l}", [F, 1], F32, isOutput=False) for l in (1, 2)]
    mw1 = nc.declare_dram_parameter("mw1", [F, 2 * F], F32, isOutput=False)
    mb1 = nc.declare_dram_parameter("mb1", [F, 2], F32, isOutput=False)
    mw2 = nc.declare_dram_parameter("mw2", [P, 2, ncls], F32, isOutput=False)
    mb2 = nc.declare_dram_parameter("mb2", [P, ncls], F32, isOutput=False)
    out = nc.declare_dram_parameter("out", [g, ncls], F32, isOutput=True)

    # ---------------- internal DRAM ----------------
    m_local = nc.dram_tensor("m_local", [npc, F], BF16)
    m_full = nc.dram_tensor("m_full", [n, F], BF16, addr_space="Shared")
    pool_part = nc.dram_tensor("pool_part", [gblocks * P, F], F32)
    pool_red = nc.dram_tensor("pool_red", [gblocks * P, F], F32, addr_space="Shared")

    cc_groups = [list(range(ncores))]

    import contextlib
    es = contextlib.ExitStack()
    # ---------------- persistent SBUF state ----------------
    hT = es.enter_context(nc.sbuf_tensor("hT", [P, npc_pad], BF16))
    idx_t = es.enter_context(nc.sbuf_tensor("idx_t", [P, C * 8], I16))
    colloc_t = es.enter_context(nc.sbuf_tensor("colloc_t", [P, C], F32))
    wvals_t = es.enter_context(nc.sbuf_tensor("wvals_t", [P, C], F32))
    batchloc_t = es.enter_context(nc.sbuf_tensor("batchloc_t", [P, nw], F32))
    growidx_t = es.enter_context(nc.sbuf_tensor("growidx_t", [P, 1], mybir.dt.int32))
    cntinv_t = es.enter_context(nc.sbuf_tensor("cntinv_t", [P, gblocks], F32))
    iota_t = es.enter_context(nc.sbuf_tensor("iota_t", [P, QW], F32))
    ident_t = es.enter_context(nc.sbuf_tensor("ident_t", [P, P], F32))
    ident_bf = es.enter_context(nc.sbuf_tensor("ident_bf", [P, P], BF16))
    wi_t = [es.enter_context(nc.sbuf_tensor(f"wi_t{l}", [F, F], BF16)) for l in range(2)]
    wr_t = [es.enter_context(nc.sbuf_tensor(f"wr_t{l}", [F, F], BF16)) for l in range(2)]
    b_t = [es.enter_context(nc.sbuf_tensor(f"b_t{l}", [F, 1], F32)) for l in range(2)]
    cc_sem = es.enter_context(nc.semaphore("cc_sem"))

    def m_pass(l, sm, psm):
        """m = h @ Wi -> m_local DRAM (node-major bf16), 8 chunks per DMA."""
        GB = 8
        for c0 in range(0, nw, GB):
            cn = min(GB, nw - c0)
            stage = sm.tile([P, GB, F], BF16, tag="mstage", name=f"mst{l}_{c0}",
                            bufs=2)
            for k in range(cn):
                ci = c0 + k
                s0, s1 = ci * P, (ci + 1) * P
                mp = psm.tile([P, F], F32, tag="mps", name=f"mp{l}_{ci}", bufs=2)
                nc.tensor.matmul(out=mp[:], lhsT=hT[:, s0:s1], rhs=wi_t[l][:],
                                 start=True, stop=True)
                nc.scalar.activation(out=stage[:, k, :], in_=mp[:], func=AF.Copy)
            rows = min(npc - c0 * P, cn * P)
            full = rows // P
            if full:
                nc.sync.dma_start(
                    out=m_local[c0 * P:c0 * P + full * P, :].rearrange(
                        "(c p) f -> p c f", p=P),
                    in_=stage[:, :full, :])
            rem = rows - full * P
            if rem:
                nc.sync.dma_start(
                    out=m_local[c0 * P + full * P:c0 * P + rows, :],
                    in_=stage[:rem, full, :])

    _reg_cache = {}

    def reg_of(val):
        if val not in _reg_cache:
            _reg_cache[val] = nc.gpsimd.to_reg(val)
        return _reg_cache[val]

    def conv_layer(l, tc, sm, msgp, ohp, psm, agg_bufs):
        # batched gathers: one dma_gather per (sb, group)
        msg_tiles = {}
        for ci_call, (sb, gi, c0, c1) in enumerate(calls):
            kb = c1 - c0
            if kb == 0:
                continue
            mt = msgp.tile([P, kbmax, F], BF16, tag=f"msg{gi}", name=f"msg{l}_{sb}_{gi}")
            nc.gpsimd.dma_gather(
                mt[:, :kb, :],
                m_full[gi * RANGE:min((gi + 1) * RANGE, n), :],
                idx_t[:, c0 * 8:c1 * 8],
                kb * P, reg_of(kb * P), F,
                single_packet=False,
            )
            msg_tiles[ci_call] = mt
        # quad loop: all matmuls for a 512-wide quad accumulate in one PSUM bank
        for q in range(nq):
            q0 = q * QW
            e0 = min(npc, q0 + QW)
            width = e0 - q0
            agg = psm.tile([F, QW], F32, tag="agg", name=f"agg{l}_{q}",
                           bufs=agg_bufs)
            nmm = 1 + sum(gq_span[(gi, q)][2] for gi in range(ngroups))
            # Wr path first (hT pad cols are zero)
            nc.tensor.matmul(out=agg[:], lhsT=wr_t[l][:],
                             rhs=hT[:, q0:q0 + QW], start=True, stop=(nmm == 1))
            j = 0
            for gi in range(ngroups):
                ci_call, loc0, k = gq_span[(gi, q)]
                if k == 0:
                    continue
                ch0 = calls[ci_call][2] + loc0
                ohq = ohp.tile([P, kgqmax, QW], BF16, tag=f"oh{gi}",
                               name=f"oh{l}_{q}_{gi}", bufs=1)
                iota_bc = iota_t[:].rearrange(
                    "p (one j) -> p one j", one=1).to_broadcast([P, k, QW])
                colloc_bc = colloc_t[:, ch0:ch0 + k].rearrange(
                    "p (c one) -> p c one", one=1).to_broadcast([P, k, QW])
                wvals_bc = wvals_t[:, ch0:ch0 + k].rearrange(
                    "p (c one) -> p c one", one=1).to_broadcast([P, k, QW])
                nc.vector.tensor_tensor(out=ohq[:, :k, :], in0=iota_bc,
                                        in1=colloc_bc, op=OP.is_equal)
                nc.vector.tensor_tensor(out=ohq[:, :k, :], in0=ohq[:, :k, :],
                                        in1=wvals_bc, op=OP.mult)
                for kk in range(k):
                    j += 1
                    nc.tensor.matmul(out=agg[:],
                                     lhsT=msg_tiles[ci_call][:, loc0 + kk, :],
                                     rhs=ohq[:, kk, :],
                                     start=False, stop=(j == nmm - 1))
            nc.scalar.activation(out=hT[:, q0:e0], in_=agg[:, :width],
                                 func=AF.Relu, bias=b_t[l][:, 0:1], scale=1.0)

    # ================= phase A: loads + m1 =================
    with tile.TileContext(nc) as tc:
        with (
            tc.tile_pool(name="sm", bufs=4) as sm,
            tc.tile_pool(name="psm", bufs=6, space="PSUM") as psm,
        ):
            nc.sync.dma_start(out=iota_t[:], in_=iota_p[:])
            nc.sync.dma_start(out=ident_t[:], in_=ident_p[:])
            nc.vector.tensor_copy(ident_bf[:], ident_t[:])
            nc.sync.dma_start(out=idx_t[:], in_=idxs_p[:])
            nc.sync.dma_start(out=colloc_t[:], in_=colloc[:])
            nc.sync.dma_start(out=wvals_t[:], in_=wvals[:])
            nc.sync.dma_start(out=batchloc_t[:], in_=batchloc[:])
            nc.sync.dma_start(out=growidx_t[:], in_=growidx[:])
            nc.sync.dma_start(out=cntinv_t[:], in_=cntinv[:])
            for l in range(2):
                nc.sync.dma_start(out=wi_t[l][:], in_=wi[l][:])
                nc.sync.dma_start(out=wr_t[l][:], in_=wr[l][:])
                nc.sync.dma_start(out=b_t[l][:], in_=bb[l][:])
            nc.sync.dma_start(out=hT[:], in_=xT[:])
            m_pass(0, sm, psm)

    # ================= conv layers (+ pooling fused into conv1) =================
    for l in range(2):
        nc.gpsimd.collective_compute(
            "AllGather", OP.bypass, replica_groups=cc_groups,
            ins=[m_local[:]], outs=[m_full[:]]).then_inc(cc_sem, 1)
        nc.gpsimd.wait_ge(cc_sem, l + 1)
        nc.sync.wait_ge(cc_sem, l + 1)

        with tile.TileContext(nc) as tc:
            with (
                tc.tile_pool(name="sm", bufs=4) as sm,
                tc.tile_pool(name="msgp", bufs=2) as msgp,
                tc.tile_pool(name="ohp", bufs=2) as ohp,
                tc.tile_pool(name="psm", bufs=6, space="PSUM") as psm,
                tc.tile_pool(name="psPool", bufs=1, space="PSUM") as psPool,
            ):
                conv_layer(l, tc, sm, msgp, ohp, psm, agg_bufs=4 if l == 0 else 3)
                if l == 0:
                    m_pass(1, sm, psm)
                else:
                    # ---- pooling: per-core local-graph sums + indirect scatter
                    zt = sm.tile([P, F], F32, tag="zt", name="zt")
                    nc.vector.memset(zt[:], 0.0)
                    for b in range(gblocks):
                        nc.sync.dma_start(out=pool_part[b * P:(b + 1) * P, :],
                                          in_=zt[:])
                    ohgb = ohp.tile([P, nw, P], BF16, tag="ohgb", name="ohgb", bufs=1)
                    iota_bc = iota_t[:, :P].rearrange("p (one j) -> p one j", one=1).to_broadcast([P, nw, P])
                    bl_bc = batchloc_t[:].rearrange("p (c one) -> p c one", one=1).to_broadcast([P, nw, P])
                    nc.vector.tensor_tensor(out=ohgb[:], in0=iota_bc, in1=bl_bc,
                                            op=OP.is_equal)
                    pool_acc = psPool.tile([P, F], F32, tag="pacc", name="pool_acc")
                    for ci in range(nw):
                        s0 = ci * P
                        tp = psm.tile([P, F], BF16, tag="tp", name=f"tp{ci}", bufs=3)
                        nc.tensor.transpose(out=tp[:], in_=hT[:, s0:s0 + P],
                                            identity=ident_bf[:])
                        nx = sm.tile([P, F], BF16, tag="nx", name=f"nx{ci}")
                        nc.scalar.activation(out=nx[:], in_=tp[:], func=AF.Copy)
                        nc.tensor.matmul(out=pool_acc[:], lhsT=ohgb[:, ci, :],
                                         rhs=nx[:],
                                         start=(ci == 0), stop=(ci == nw - 1))
                    pool_loc = sm.tile([P, F], F32, tag="ploc", name="pool_loc")
                    nc.scalar.activation(out=pool_loc[:], in_=pool_acc[:],
                                         func=AF.Copy)
                    nc.gpsimd.indirect_dma_start(
                        out=pool_part[:],
                        out_offset=bass.IndirectOffsetOnAxis(
                            ap=growidx_t[:, 0:1], axis=0),
                        in_=pool_loc[:], in_offset=None,
                        bounds_check=g - 1, oob_is_err=False)

    nc.gpsimd.collective_compute(
        "AllReduce", OP.add, replica_groups=cc_groups,
        ins=[pool_part[:]], outs=[pool_red[:]]).then_inc(cc_sem, 1)
    nc.sync.wait_ge(cc_sem, 3)

    # ================= mean + MLP head =================
    with tile.TileContext(nc) as tc:
        with (
            tc.tile_pool(name="sm", bufs=4) as sm,
            tc.tile_pool(name="one", bufs=1) as one,
            tc.tile_pool(name="psm", bufs=4, space="PSUM") as psm,
        ):
            meanT = one.tile([F, gblocks * P], F32)
            for b in range(gblocks):
                pr = sm.tile([P, F], F32, tag="pr", name=f"pr{b}")
                nc.sync.dma_start(out=pr[:], in_=pool_red[b * P:(b + 1) * P, :])
                mg = sm.tile([P, F], F32, tag="mg", name=f"mg{b}")
                nc.vector.tensor_scalar(out=mg[:], in0=pr[:],
                                        scalar1=cntinv_t[:, b:b + 1],
                                        scalar2=None, op0=OP.mult)
                mt = psm.tile([F, P], F32, tag="ps", name=f"mt{b}", bufs=2)
                nc.tensor.transpose(out=mt[:], in_=mg[:], identity=ident_t[:])
                nc.scalar.activation(out=meanT[:, b * P:(b + 1) * P], in_=mt[:],
                                     func=AF.Copy)
            mw1_t = one.tile([F, 2 * F], F32)
            nc.sync.dma_start(out=mw1_t[:], in_=mw1[:])
            mb1_t = one.tile([F, 2], F32)
            nc.sync.dma_start(out=mb1_t[:], in_=mb1[:])
            mw2_t = one.tile([P, 2, ncls], F32)
            nc.sync.dma_start(out=mw2_t[:], in_=mw2[:])
            mb2_t = one.tile([P, ncls], F32)
            nc.sync.dma_start(out=mb2_t[:], in_=mb2[:])
            hidT = one.tile([F, 2, gblocks * P], F32)
            for hc in range(2):
                hps = psm.tile([F, gblocks * P], F32, tag="hps", name=f"hps{hc}", bufs=2)
                nc.tensor.matmul(out=hps[:], lhsT=mw1_t[:, hc * F:(hc + 1) * F],
                                 rhs=meanT[:], start=True, stop=True)
                nc.scalar.activation(out=hidT[:, hc, :], in_=hps[:], func=AF.Relu,
                                     bias=mb1_t[:, hc:hc + 1], scale=1.0)
            p_out = min(P, g)
            outsb = one.tile([P, gblocks, ncls], F32)
            for gc in range(gblocks):
                ops_ = psm.tile([P, ncls], F32, tag="ps", name=f"ops{gc}", bufs=2)
                for hc in range(2):
                    nc.tensor.matmul(out=ops_[:], lhsT=hidT[:, hc, gc * P:(gc + 1) * P],
                                     rhs=mw2_t[:, hc, :],
                                     start=(hc == 0), stop=(hc == 1))
                nc.vector.tensor_tensor(out=outsb[:, gc, :], in0=ops_[:],
                                        in1=mb2_t[:], op=OP.add)
            nc.sync.dma_start(
                out=out.rearrange("(b p) c -> p b c", p=p_out),
                in_=outsb[:p_out, :, :])

    es.close()

    # SWDGE Q7 library load for InstDMAGatherAnt + ISA codegen
    import concourse.bacc as bacc
    bacc.Bacc.insert_library_loads(nc)
    mybir.codegen_inst_isa_subclasses(nc)
    return nc


def make_inputs(pre, x, Wi1, Wr1, b1, Wi2, Wr2, b2, mW1, mb1, mW2, mb2,
                n, ncores, g, ncls):
    """Build per-core in_maps."""
    npc = pre["npc"]
    npc_pad = pre["npc_pad"]
    iota = np.tile(np.arange(pre["QW"], dtype=np.float32)[None, :], (P, 1))
    ident = np.eye(P, dtype=np.float32)
    x = np.asarray(x, np.float32)
    in_maps = []
    mb1w = np.ascontiguousarray(np.asarray(mb1, np.float32).reshape(2, P).T)
    mb2r = np.tile(np.asarray(mb2, np.float32).reshape(1, ncls), (P, 1))
    for c in range(ncores):
        xs = np.zeros((P, npc_pad), np.float32)
        xs[:, :npc] = x[c * npc:(c + 1) * npc, :].T
        m = dict(
            xT=to_bf16(xs),
            idxs=pre["cores"][c]["idxs"],
            colloc=pre["cores"][c]["colloc"],
            wvals=pre["cores"][c]["wvals"],
            batchloc=pre["cores"][c]["batchloc"],
            growidx=pre["cores"][c]["growidx"],
            cntinv=pre["cntinv"],
            iota512=iota,
            ident128=ident,
            wi1=to_bf16(Wi1), wr1=to_bf16(Wr1),
            wi2=to_bf16(Wi2), wr2=to_bf16(Wr2),
            b1=np.asarray(b1, np.float32).reshape(P, 1),
            b2=np.asarray(b2, np.float32).reshape(P, 1),
            mw1=np.asarray(mW1, np.float32),
            mb1=mb1w,
            mw2=np.ascontiguousarray(
                np.asarray(mW2, np.float32).reshape(2, P, ncls).transpose(1, 0, 2)),
            mb2=mb2r,
        )
        in_maps.append(m)
    return in_maps


# ======================= entry point =======================
N_FULL = 100000
E_FULL = 640000
G_FULL = 512
NCLS_FULL = 2
NCORES = 8

_cache = {}


def kernel(x, edge_index, edge_attr, batch, Wi1, Wr1, b1, Wi2, Wr2, b2,
           mW1, mb1, mW2, mb2):
    install()
    x = np.asarray(x)
    edge_index = np.asarray(edge_index)
    edge_attr = np.asarray(edge_attr)
    batch = np.asarray(batch)
    n, f = x.shape
    g = G_FULL
    ncls = np.asarray(mW2).shape[1]

    pre = preprocess(edge_index, edge_attr, batch, n, NCORES, g)
    key = (n, g, ncls, pre["C"])
    if key not in _cache:
        nc = build_nc(pre, n, NCORES, g, ncls)
        _cache[key] = SpmdKernel(nc)
    k = _cache[key]
    in_maps = make_inputs(pre, x, Wi1, Wr1, b1, Wi2, Wr2, b2,
                          mW1, mb1, mW2, mb2, n, NCORES, g, ncls)
    ci, zz = k.put_inputs(in_maps)
    res = k.run_np(ci, zz)
    return np.ascontiguousarray(res[0]["out"].astype(np.float32))


# revision 38
# speedup vs baseline: 1.1647x; 1.1647x over previous
"""Self-contained Trainium2 Bass kernel for the ARMA GNN problem
(nn_ARMA_49297634623854).

kernel(**inputs) takes the FULL unsharded inputs (x, edge_index, edge_attr,
batch, Wi1, Wr1, b1, Wi2, Wr2, b2, mW1, mb1, mW2, mb2) as numpy arrays,
shards node-contiguously across 8 NeuronCores, runs an SPMD Bass kernel
(batched dma_gather edge gather + one-hot-matmul scatter fused in PSUM +
AllGather/AllReduce collectives), and returns the full [512, 2] float32
output.

Design notes:
- gcn_norm (deg/dinv) is computed on the host and baked into per-edge
  weights, so the device never computes degrees or rescales messages.
- Edge messages m = h @ Wi are AllGathered (bf16), then gathered per-edge
  with batched dma_gather (int16 indices -> 4 source-range groups of 25000
  rows each).
- Edges are grouped by (source-range group, 128-wide dest window); for each
  window all scatter matmuls plus the Wr-path matmul accumulate into one
  PSUM tile, and a single scalar-engine activation (relu + bias) writes the
  new hT window. No aggregation buffer, no vector adds.
"""

# ======================= walrus wait-splitting patches =======================
import concourse.mybir as mybir
import concourse.tile as tile
from concourse.vector_clock import ScopedClock, VectorClock

_nop_counter = [0]


def _make_wait_nop(engine, wait):
    _nop_counter[0] += 1
    return mybir.InstNoOp(
        name=f"SplitWait-{_nop_counter[0]}",
        engine=engine,
        ins=[],
        outs=[],
        sync_info=mybir.SyncInfo(on_wait=[wait], on_update=[]),
        bass_nofuse=True,
    )


def _split_multi_waits(insts):
    out = []
    n_split = 0
    for inst in insts:
        si = inst.sync_info
        if si is not None and len(si.on_wait) > 1:
            waits = list(si.on_wait)
            for w in waits[:-1]:
                out.append(_make_wait_nop(inst.engine, w))
            inst.sync_info = mybir.SyncInfo(
                on_wait=[waits[-1]], on_update=list(si.on_update)
            )
            n_split += 1
        out.append(inst)
    return out, n_split


_orig_lower = tile.TileContext._lower_ordered_insts


def _patched_lower(self, postordered_blocks):
    total = 0
    for bbname in list(postordered_blocks.keys()):
        newlist, n = _split_multi_waits(postordered_blocks[bbname])
        postordered_blocks[bbname] = newlist
        total += n
    return _orig_lower(self, postordered_blocks)


def _patched_drain_and_barrier(self, tick_clock, wait_clock):
    gc = tick_clock.global_clock
    nprocs = len(gc)
    for p in range(nprocs):
        t = gc[p]
        if t <= 0:
            continue
        vec = [0] * nprocs
        vec[p] = t
        nop_inst = self.nc.sync.nop(nofuse=True)
        wait_clock.add_sem_waits(nop_inst.ins, ScopedClock({None: VectorClock(vec)}))
    self.nc.sync.drain()
    self.nc.all_engine_barrier()
    assert self.sems is not None
    popped = self.nc._tile_sem_poison_stack.pop()
    assert popped is self._sem_poison
    self.nc.clear_and_free_semaphores(list(self.sems.allocated().values()))
    self.nc.all_engine_barrier()


def install():
    tile.TileContext._lower_ordered_insts = _patched_lower
    tile.TileContext._drain_and_barrier = _patched_drain_and_barrier


# ======================= SPMD runner =======================
import time

import jax
import numpy as np
from jax.sharding import Mesh, NamedSharding, PartitionSpec
from jax.experimental.shard_map import shard_map

import concourse.bass as bass
import concourse.mybir as mybir
from concourse import bass2jax
from concourse.bass2jax import (
    _bass_exec_p,
    fast_dispatch_compile,
    install_neuronx_cc_hook,
    partition_id_tensor,
)


class SpmdKernel:
    def __init__(self, nc: bass.Bass, n_cores: int = 8):
        install_neuronx_cc_hook()
        self.nc = nc
        self.n_cores = n_cores
        in_names: list[str] = []
        out_names: list[str] = []
        out_avals: list[jax.core.ShapedArray] = []
        partition_name = (
            nc.partition_id_tensor.name if nc.partition_id_tensor else None
        )
        for alloc in nc.m.functions[0].allocations:
            if not isinstance(alloc, mybir.MemoryLocationSet):
                continue
            name = alloc.memorylocations[0].name
            if alloc.kind == "ExternalInput":
                if name != partition_name:
                    in_names.append(name)
            elif alloc.kind == "ExternalOutput":
                shape = tuple(alloc.tensor_shape)
                dtype = mybir.dt.np(alloc.dtype)
                out_names.append(name)
                out_avals.append(jax.core.ShapedArray(shape, dtype))
        self.n_params = len(in_names)
        self.out_names = out_names
        self.out_avals = out_avals
        self.in_names = in_names[:]
        all_in_names = in_names + out_names
        if partition_name is not None:
            all_in_names.append(partition_name)

        def _body(*args):
            operands = list(args)
            if partition_name is not None:
                operands.append(partition_id_tensor())
            outs = _bass_exec_p.bind(
                *operands,
                out_avals=tuple(out_avals),
                in_names=tuple(all_in_names),
                out_names=tuple(out_names),
                lowering_input_output_aliases=(),
                sim_require_finite=True,
                sim_require_nnan=True,
                nc=nc,
            )
            return tuple(outs)

        devices = jax.devices()[: n_cores]
        assert len(devices) == n_cores
        self.mesh = Mesh(np.asarray(devices), ("core",))
        n_out = len(out_names)
        in_specs = (PartitionSpec("core"),) * (self.n_params + n_out)
        out_specs = (PartitionSpec("core"),) * n_out
        self._sharded = shard_map(
            _body,
            mesh=self.mesh,
            in_specs=in_specs,
            out_specs=out_specs,
            check_rep=False,
        )
        self.fn = jax.jit(self._sharded, keep_unused=True)
        self._compiled = None
        self.sharding = NamedSharding(self.mesh, PartitionSpec("core"))

    def compile_fast(self, concat_in, zeros):
        """AOT compile with fast dispatch (no effects)."""
        self._compiled = fast_dispatch_compile(
            lambda: jax.jit(self._sharded, keep_unused=True)
            .lower(*concat_in, *zeros)
            .compile()
        )
        return self._compiled

    def put_inputs(self, in_maps: list[dict[str, np.ndarray]]):
        """in_maps: one dict per core. Returns list of device arrays (concat
        along axis 0) in in_names order, plus zero output buffers."""
        concat_in = []
        for name in self.in_names:
            arrs = [np.asarray(in_maps[c][name]) for c in range(self.n_cores)]
            concat_in.append(
                jax.device_put(np.concatenate(arrs, axis=0), self.sharding)
            )
        zeros = []
        for av in self.out_avals:
            z = np.zeros((self.n_cores * av.shape[0], *av.shape[1:]), av.dtype)
            zeros.append(jax.device_put(z, self.sharding))
        return concat_in, zeros

    def __call__(self, concat_in, zeros):
        f = self._compiled or self.fn
        outs = f(*concat_in, *zeros)
        return outs

    def run_np(self, concat_in, zeros):
        f = self._compiled or self.fn
        outs = f(*concat_in, *zeros)
        res = []
        for c in range(self.n_cores):
            res.append(
                {
                    name: np.asarray(outs[i]).reshape(
                        self.n_cores, *self.out_avals[i].shape
                    )[c]
                    for i, name in enumerate(self.out_names)
                }
            )
        return res

    def time_it(self, concat_in, zeros, reps=20, warmup=3):
        f = self._compiled or self.fn
        for _ in range(warmup):
            jax.block_until_ready(f(*concat_in, *zeros))
        ts = []
        for _ in range(reps):
            t0 = time.perf_counter()
            jax.block_until_ready(f(*concat_in, *zeros))
            ts.append(time.perf_counter() - t0)
        return min(ts), sorted(ts)[len(ts) // 2]


# ======================= GNN kernel builder =======================
import concourse.bass as bass
import concourse.mybir as mybir
import concourse.tile as tile

F32 = mybir.dt.float32
BF16 = mybir.dt.bfloat16
I16 = mybir.dt.int16
AF = mybir.ActivationFunctionType
OP = mybir.AluOpType
P = 128
RANGE = 25000  # int16-safe dma_gather source range
NSB = 8        # gather superblocks per layer per group


def wrap128(arr):
    """[C*128,...] -> [128, C] layout: out[p, c] = arr[c*128+p]."""
    C = arr.shape[0] // 128
    return np.ascontiguousarray(arr.reshape(C, 128).T)


def to_bf16(a):
    import jax.numpy as jnp
    return np.asarray(jnp.asarray(np.asarray(a, np.float32), dtype=jnp.bfloat16))


def preprocess(edge_index, edge_attr, batch, n, ncores, g):
    """Host-side: gcn_norm, per-core edge->chunk tables, pooling tables.

    Chunks are 128 edge slots grouped by (source-range group, 512-wide dest
    quad). Chunk ordering: for sb: for group: for quad in sb: chunks -- so
    each (sb, group) is one contiguous dma_gather call span.
    """
    npc = n // ncores
    nw = (npc + 127) // 128          # 128-node chunks (pooling, m-pass)
    QW = 128                          # scatter window width
    nq = (npc + QW - 1) // QW
    npc_pad = nq * QW                 # hT padded to whole quads
    ngroups = (n + RANGE - 1) // RANGE
    gblocks = (g + P - 1) // P

    row = np.asarray(edge_index[0], np.int64)
    col = np.asarray(edge_index[1], np.int64)
    w_attr = np.asarray(edge_attr, np.float32).reshape(-1)
    batch = np.asarray(batch, np.int64)

    deg = np.bincount(col, weights=w_attr.astype(np.float64), minlength=n)
    dinv = np.where(deg > 0, 1.0 / np.sqrt(np.maximum(deg, 1e-12)), 0.0)
    norm = (dinv[row] * w_attr * dinv[col]).astype(np.float32)

    per_core = []
    cnts = np.zeros((ncores, ngroups, nq), np.int64)
    for c in range(ncores):
        m = (col // npc) == c
        r_c = row[m]
        d_c = col[m] - c * npc
        n_c = norm[m]
        g_c = r_c // RANGE
        q_c = d_c // QW
        o = np.lexsort((r_c, q_c, g_c))
        r_c, d_c, n_c, g_c, q_c = r_c[o], d_c[o], n_c[o], g_c[o], q_c[o]
        per_core.append((r_c, d_c, n_c, g_c, q_c))
        for gi in range(ngroups):
            cnts[c, gi] = np.bincount(q_c[g_c == gi], minlength=nq)
    nch = ((cnts.max(axis=0) + P - 1) // P).astype(np.int64)  # [ngroups, nq]

    # quad -> superblock (balanced, monotone)
    qsb = (np.arange(nq) * NSB) // nq
    chunk_of = {}
    calls = []  # (sb, g, c0, c1)
    gq_span = {}  # (g, q) -> (call_idx, local0, k)
    C = 0
    for sb in range(NSB):
        qs = [q for q in range(nq) if qsb[q] == sb]
        for gi in range(ngroups):
            c0 = C
            ci_call = len(calls)
            for q in qs:
                chunk_of[(gi, q)] = C
                gq_span[(gi, q)] = (ci_call, C - c0, int(nch[gi, q]))
                C += int(nch[gi, q])
            calls.append((sb, gi, c0, C))
    kbmax = max(c1 - c0 for (_, _, c0, c1) in calls)
    kgqmax = int(nch.max())

    cores = []
    for c in range(ncores):
        r_c, d_c, n_c, g_c, q_c = per_core[c]
        idx16 = np.zeros(C * P, np.int16)
        colloc = np.zeros(C * P, np.float32)
        wvals = np.zeros(C * P, np.float32)
        key = g_c * nq + q_c
        uniq, starts = np.unique(key, return_index=True)
        ends = np.r_[starts[1:], len(key)]
        for u, s0, s1 in zip(uniq, starts, ends):
            gi, q = divmod(int(u), nq)
            base = chunk_of[(gi, q)] * P
            cnt = int(s1 - s0)
            sl = slice(base, base + cnt)
            idx16[sl] = (r_c[s0:s1] - gi * RANGE).astype(np.int16)
            colloc[sl] = (d_c[s0:s1] - q * QW).astype(np.float32)
            wvals[sl] = n_c[s0:s1]
        idx_t = np.tile(np.ascontiguousarray(idx16.reshape(C * 8, 16).T), (8, 1))
        cores.append(
            dict(
                idxs=np.ascontiguousarray(idx_t),
                colloc=wrap128(colloc),
                wvals=wrap128(wvals),
            )
        )

    # pooling: per-core local graph index (slab spans <= 128 graphs)
    for c in range(ncores):
        bl = batch[c * npc:(c + 1) * npc]
        g0, g1 = int(bl[0]), int(bl[-1])
        assert g1 - g0 + 1 <= P, f"core {c} graph span {g1 - g0 + 1} > 128"
        blloc = np.full(nw * P, -1.0, np.float32)
        blloc[:npc] = (bl - g0).astype(np.float32)
        cores[c]["batchloc"] = wrap128(blloc)  # [P, nw]
        cores[c]["growidx"] = (g0 + np.arange(P, dtype=np.int32)).reshape(P, 1)

    cnt = np.bincount(batch, minlength=g).astype(np.float64)
    cnt_inv = (1.0 / np.maximum(cnt, 1.0)).astype(np.float32)  # [g]
    cntinv_t = np.ascontiguousarray(cnt_inv.reshape(gblocks, P).T)  # [P, gblocks]

    return dict(
        C=C, calls=calls, gq_span=gq_span, kbmax=kbmax, kgqmax=kgqmax,
        nw=nw, nq=nq, QW=QW, npc=npc, npc_pad=npc_pad, ngroups=ngroups,
        gblocks=gblocks, cores=cores, cntinv=cntinv_t,
    )


def build_nc(pre, n, ncores, g, ncls):
    F = 128
    C = pre["C"]
    nw = pre["nw"]
    nq = pre["nq"]
    QW = pre["QW"]
    npc = pre["npc"]
    npc_pad = pre["npc_pad"]
    ngroups = pre["ngroups"]
    gblocks = pre["gblocks"]
    calls = pre["calls"]
    gq_span = pre["gq_span"]
    kbmax = pre["kbmax"]
    kgqmax = pre["kgqmax"]
    # descriptor-ring capacity check: ~255 descs/engine-ring with 32KB scratch
    assert kbmax * P // 16 + 1 <= 248, f"gather call too big: kbmax={kbmax}"

    nc = bass.Bass(dynamic_dma_scratch_size=32768)

    # ---------------- parameters ----------------
    xT = nc.declare_dram_parameter("xT", [P, npc_pad], BF16, isOutput=False)
    idxs_p = nc.declare_dram_parameter("idxs", [P, C * 8], I16, isOutput=False)
    colloc = nc.declare_dram_parameter("colloc", [P, C], BF16, isOutput=False)
    wvals = nc.declare_dram_parameter("wvals", [P, C], BF16, isOutput=False)
    batchloc = nc.declare_dram_parameter("batchloc", [P, nw], BF16, isOutput=False)
    growidx = nc.declare_dram_parameter("growidx", [P, 1], mybir.dt.int32, isOutput=False)
    cntinv = nc.declare_dram_parameter("cntinv", [P, gblocks], F32, isOutput=False)
    iota_p = nc.declare_dram_parameter("iota512", [P, QW], BF16, isOutput=False)
    ident_p = nc.declare_dram_parameter("ident128", [P, P], F32, isOutput=False)
    wi = [nc.declare_dram_parameter(f"wi{l}", [F, F], BF16, isOutput=False) for l in (1, 2)]
    wr = [nc.declare_dram_parameter(f"wr{l}", [F, F], BF16, isOutput=False) for l in (1, 2)]
    bb = [nc.declare_dram_parameter(f"b{l}", [F, 1], F32, isOutput=False) for l in (1, 2)]
    mw1 = nc.declare_dram_parameter("mw1", [F, 2 * F], F32, isOutput=False)
    mb1 = nc.declare_dram_parameter("mb1", [F, 2], F32, isOutput=False)
    mw2 = nc.declare_dram_parameter("mw2", [P, 2, ncls], F32, isOutput=False)
    mb2 = nc.declare_dram_parameter("mb2", [P, ncls], F32, isOutput=False)
    out = nc.declare_dram_parameter("out", [g, ncls], F32, isOutput=True)

    # ---------------- internal DRAM ----------------
    m_local = nc.dram_tensor("m_local", [npc, F], BF16)
    m_full = nc.dram_tensor("m_full", [n, F], BF16, addr_space="Shared")
    pool_part = nc.dram_tensor("pool_part", [gblocks * P, F], F32)
    pool_red = nc.dram_tensor("pool_red", [gblocks * P, F], F32, addr_space="Shared")

    cc_groups = [list(range(ncores))]

    import contextlib
    es = contextlib.ExitStack()
    # ---------------- persistent SBUF state ----------------
    hT = es.enter_context(nc.sbuf_tensor("hT", [P, npc_pad], BF16))
    idx_t = es.enter_context(nc.sbuf_tensor("idx_t", [P, C * 8], I16))
    colloc_t = es.enter_context(nc.sbuf_tensor("colloc_t", [P, C], BF16))
    wvals_t = es.enter_context(nc.sbuf_tensor("wvals_t", [P, C], BF16))
    batchloc_t = es.enter_context(nc.sbuf_tensor("batchloc_t", [P, nw], BF16))
    growidx_t = es.enter_context(nc.sbuf_tensor("growidx_t", [P, 1], mybir.dt.int32))
    cntinv_t = es.enter_context(nc.sbuf_tensor("cntinv_t", [P, gblocks], F32))
    iota_t = es.enter_context(nc.sbuf_tensor("iota_t", [P, QW], BF16))
    ident_t = es.enter_context(nc.sbuf_tensor("ident_t", [P, P], F32))
    ident_bf = es.enter_context(nc.sbuf_tensor("ident_bf", [P, P], BF16))
    wi_t = [es.enter_context(nc.sbuf_tensor(f"wi_t{l}", [F, F], BF16)) for l in range(2)]
    wr_t = [es.enter_context(nc.sbuf_tensor(f"wr_t{l}", [F, F], BF16)) for l in range(2)]
    b_t = [es.enter_context(nc.sbuf_tensor(f"b_t{l}", [F, 1], F32)) for l in range(2)]
    cc_sem = es.enter_context(nc.semaphore("cc_sem"))

    def m_pass(l, sm, psm):
        """m = h @ Wi -> m_local DRAM (node-major bf16), 8 chunks per DMA."""
        GB = 8
        for c0 in range(0, nw, GB):
            cn = min(GB, nw - c0)
            stage = sm.tile([P, GB, F], BF16, tag="mstage", name=f"mst{l}_{c0}",
                            bufs=2)
            for k in range(cn):
                ci = c0 + k
                s0, s1 = ci * P, (ci + 1) * P
                mp = psm.tile([P, F], F32, tag="mps", name=f"mp{l}_{ci}", bufs=2)
                nc.tensor.matmul(out=mp[:], lhsT=hT[:, s0:s1], rhs=wi_t[l][:],
                                 start=True, stop=True)
                nc.scalar.activation(out=stage[:, k, :], in_=mp[:], func=AF.Copy)
            rows = min(npc - c0 * P, cn * P)
            full = rows // P
            if full:
                nc.sync.dma_start(
                    out=m_local[c0 * P:c0 * P + full * P, :].rearrange(
                        "(c p) f -> p c f", p=P),
                    in_=stage[:, :full, :])
            rem = rows - full * P
            if rem:
                nc.sync.dma_start(
                    out=m_local[c0 * P + full * P:c0 * P + rows, :],
                    in_=stage[:rem, full, :])

    _reg_cache = {}

    def reg_of(val):
        if val not in _reg_cache:
            _reg_cache[val] = nc.gpsimd.to_reg(val)
        return _reg_cache[val]

    def conv_layer(l, tc, sm, msgp, ohp, psm, agg_bufs):
        # batched gathers + batched one-hot gen per (sb, group)
        msg_tiles = {}
        oh_tiles = {}
        for ci_call, (sb, gi, c0, c1) in enumerate(calls):
            kb = c1 - c0
            if kb == 0:
                continue
            mt = msgp.tile([P, kbmax, F], BF16, tag=f"msg{gi}", name=f"msg{l}_{sb}_{gi}")
            nc.gpsimd.dma_gather(
                mt[:, :kb, :],
                m_full[gi * RANGE:min((gi + 1) * RANGE, n), :],
                idx_t[:, c0 * 8:c1 * 8],
                kb * P, reg_of(kb * P), F,
                single_packet=False,
            )
            msg_tiles[ci_call] = mt
            ohb = ohp.tile([P, kbmax, QW], BF16, tag=f"ohb{gi}",
                           name=f"ohb{l}_{sb}_{gi}", bufs=2)
            iota_bc = iota_t[:].rearrange(
                "p (one j) -> p one j", one=1).to_broadcast([P, kb, QW])
            colloc_bc = colloc_t[:, c0:c1].rearrange(
                "p (c one) -> p c one", one=1).to_broadcast([P, kb, QW])
            wvals_bc = wvals_t[:, c0:c1].rearrange(
                "p (c one) -> p c one", one=1).to_broadcast([P, kb, QW])
            nc.vector.tensor_tensor(out=ohb[:, :kb, :], in0=iota_bc,
                                    in1=colloc_bc, op=OP.is_equal)
            nc.vector.tensor_tensor(out=ohb[:, :kb, :], in0=ohb[:, :kb, :],
                                    in1=wvals_bc, op=OP.mult)
            oh_tiles[ci_call] = ohb
        # window loop: all matmuls for a window accumulate in one PSUM tile
        for q in range(nq):
            q0 = q * QW
            e0 = min(npc, q0 + QW)
            width = e0 - q0
            agg = psm.tile([F, QW], F32, tag="agg", name=f"agg{l}_{q}",
                           bufs=agg_bufs)
            nmm = 1 + sum(gq_span[(gi, q)][2] for gi in range(ngroups))
            # Wr path first (hT pad cols are zero)
            nc.tensor.matmul(out=agg[:], lhsT=wr_t[l][:],
                             rhs=hT[:, q0:q0 + QW], start=True, stop=(nmm == 1))
            j = 0
            for gi in range(ngroups):
                ci_call, loc0, k = gq_span[(gi, q)]
                if k == 0:
                    continue
                for kk in range(k):
                    j += 1
                    nc.tensor.matmul(out=agg[:],
                                     lhsT=msg_tiles[ci_call][:, loc0 + kk, :],
                                     rhs=oh_tiles[ci_call][:, loc0 + kk, :],
                                     start=False, stop=(j == nmm - 1))
            nc.scalar.activation(out=hT[:, q0:e0], in_=agg[:, :width],
                                 func=AF.Relu, bias=b_t[l][:, 0:1], scale=1.0)

    # ================= phase A: loads + m1 =================
    with tile.TileContext(nc) as tc:
        with (
            tc.tile_pool(name="sm", bufs=4) as sm,
            tc.tile_pool(name="psm", bufs=6, space="PSUM") as psm,
        ):
            nc.sync.dma_start(out=iota_t[:], in_=iota_p[:])
            nc.sync.dma_start(out=ident_t[:], in_=ident_p[:])
            nc.vector.tensor_copy(ident_bf[:], ident_t[:])
            nc.sync.dma_start(out=idx_t[:], in_=idxs_p[:])
            nc.sync.dma_start(out=colloc_t[:], in_=colloc[:])
            nc.sync.dma_start(out=wvals_t[:], in_=wvals[:])
            nc.sync.dma_start(out=batchloc_t[:], in_=batchloc[:])
            nc.sync.dma_start(out=growidx_t[:], in_=growidx[:])
            nc.sync.dma_start(out=cntinv_t[:], in_=cntinv[:])
            for l in range(2):
                nc.sync.dma_start(out=wi_t[l][:], in_=wi[l][:])
                nc.sync.dma_start(out=wr_t[l][:], in_=wr[l][:])
                nc.sync.dma_start(out=b_t[l][:], in_=bb[l][:])
            nc.sync.dma_start(out=hT[:], in_=xT[:])
            m_pass(0, sm, psm)

    # ================= conv layers (+ pooling fused into conv1) =================
    for l in range(2):
        nc.gpsimd.collective_compute(
            "AllGather", OP.bypass, replica_groups=cc_groups,
            ins=[m_local[:]], outs=[m_full[:]]).then_inc(cc_sem, 1)
        nc.gpsimd.wait_ge(cc_sem, l + 1)
        nc.sync.wait_ge(cc_sem, l + 1)

        with tile.TileContext(nc) as tc:
            with (
                tc.tile_pool(name="sm", bufs=4) as sm,
                tc.tile_pool(name="msgp", bufs=2) as msgp,
                tc.tile_pool(name="ohp", bufs=2) as ohp,
                tc.tile_pool(name="psm", bufs=6, space="PSUM") as psm,
                tc.tile_pool(name="psPool", bufs=1, space="PSUM") as psPool,
            ):
                conv_layer(l, tc, sm, msgp, ohp, psm, agg_bufs=4 if l == 0 else 3)
                if l == 0:
                    m_pass(1, sm, psm)
                else:
                    # ---- pooling: per-core local-graph sums + indirect scatter
                    zt = sm.tile([P, F], F32, tag="zt", name="zt")
                    nc.vector.memset(zt[:], 0.0)
                    for b in range(gblocks):
                        nc.sync.dma_start(out=pool_part[b * P:(b + 1) * P, :],
                                          in_=zt[:])
                    ohgb = ohp.tile([P, nw, P], BF16, tag="ohgb", name="ohgb", bufs=1)
                    iota_bc = iota_t[:, :P].rearrange("p (one j) -> p one j", one=1).to_broadcast([P, nw, P])
                    bl_bc = batchloc_t[:].rearrange("p (c one) -> p c one", one=1).to_broadcast([P, nw, P])
                    nc.vector.tensor_tensor(out=ohgb[:], in0=iota_bc, in1=bl_bc,
                                            op=OP.is_equal)
                    pool_acc = psPool.tile([P, F], F32, tag="pacc", name="pool_acc")
                    for ci in range(nw):
                        s0 = ci * P
                        tp = psm.tile([P, F], BF16, tag="tp", name=f"tp{ci}", bufs=3)
                        nc.tensor.transpose(out=tp[:], in_=hT[:, s0:s0 + P],
                                            identity=ident_bf[:])
                        nx = sm.tile([P, F], BF16, tag="nx", name=f"nx{ci}")
                        nc.scalar.activation(out=nx[:], in_=tp[:], func=AF.Copy)
                        nc.tensor.matmul(out=pool_acc[:], lhsT=ohgb[:, ci, :],
                                         rhs=nx[:],
                                         start=(ci == 0), stop=(ci == nw - 1))
                    pool_loc = sm.tile([P, F], F32, tag="ploc", name="pool_loc")
                    nc.scalar.activation(out=pool_loc[:], in_=pool_acc[:],
                                         func=AF.Copy)
                    nc.gpsimd.indirect_dma_start(
                        out=pool_part[:],
                        out_offset=bass.IndirectOffsetOnAxis(
                            ap=growidx_t[:, 0:1], axis=0),
                        in_=pool_loc[:], in_offset=None,
                        bounds_check=g - 1, oob_is_err=False)

    nc.gpsimd.collective_compute(
        "AllReduce", OP.add, replica_groups=cc_groups,
        ins=[pool_part[:]], outs=[pool_red[:]]).then_inc(cc_sem, 1)
    nc.sync.wait_ge(cc_sem, 3)

    # ================= mean + MLP head =================
    with tile.TileContext(nc) as tc:
        with (
            tc.tile_pool(name="sm", bufs=4) as sm,
            tc.tile_pool(name="one", bufs=1) as one,
            tc.tile_pool(name="psm", bufs=4, space="PSUM") as psm,
        ):
            meanT = one.tile([F, gblocks * P], F32)
            for b in range(gblocks):
                pr = sm.tile([P, F], F32, tag="pr", name=f"pr{b}")
                nc.sync.dma_start(out=pr[:], in_=pool_red[b * P:(b + 1) * P, :])
                mg = sm.tile([P, F], F32, tag="mg", name=f"mg{b}")
                nc.vector.tensor_scalar(out=mg[:], in0=pr[:],
                                        scalar1=cntinv_t[:, b:b + 1],
                                        scalar2=None, op0=OP.mult)
                mt = psm.tile([F, P], F32, tag="ps", name=f"mt{b}", bufs=2)
                nc.tensor.transpose(out=mt[:], in_=mg[:], identity=ident_t[:])
                nc.scalar.activation(out=meanT[:, b * P:(b + 1) * P], in_=mt[:],
                                     func=AF.Copy)
            mw1_t = one.tile([F, 2 * F], F32)
            nc.sync.dma_start(out=mw1_t[:], in_=mw1[:])
            mb1_t = one.tile([F, 2], F32)
            nc.sync.dma_start(out=mb1_t[:], in_=mb1[:])
            mw2_t = one.tile([P, 2, ncls], F32)
            nc.sync.dma_start(out=mw2_t[:], in_=mw2[:])
            mb2_t = one.tile([P, ncls], F32)
            nc.sync.dma_start(out=mb2_t[:], in_=mb2[:])
            hidT = one.tile([F, 2, gblocks * P], F32)
            for hc in range(2):
                hps = psm.tile([F, gblocks * P], F32, tag="hps", name=f"hps{hc}", bufs=2)
                nc.tensor.matmul(out=hps[:], lhsT=mw1_t[:, hc * F:(hc + 1) * F],
                                 rhs=meanT[:], start=True, stop=True)
                nc.scalar.activation(out=hidT[:, hc, :], in_=hps[:], func=AF.Relu,
                                     bias=mb1_t[:, hc:hc + 1], scale=1.0)
            p_out = min(P, g)
            outsb = one.tile([P, gblocks, ncls], F32)
            for gc in range(gblocks):
                ops_ = psm.tile([P, ncls], F32, tag="ps", name=f"ops{gc}", bufs=2)
                for hc in range(2):
                    nc.tensor.matmul(out=ops_[:], lhsT=hidT[:, hc, gc * P:(gc + 1) * P],
                                     rhs=mw2_t[:, hc, :],
                                     start=(hc == 0), stop=(hc == 1))
                nc.vector.tensor_tensor(out=outsb[:, gc, :], in0=ops_[:],
                                        in1=mb2_t[:], op=OP.add)
            nc.sync.dma_start(
                out=out.rearrange("(b p) c -> p b c", p=p_out),
                in_=outsb[:p_out, :, :])

    es.close()

    # SWDGE Q7 library load for InstDMAGatherAnt + ISA codegen
    import concourse.bacc as bacc
    bacc.Bacc.insert_library_loads(nc)
    mybir.codegen_inst_isa_subclasses(nc)
    return nc


def make_inputs(pre, x, Wi1, Wr1, b1, Wi2, Wr2, b2, mW1, mb1, mW2, mb2,
                n, ncores, g, ncls):
    """Build per-core in_maps."""
    npc = pre["npc"]
    npc_pad = pre["npc_pad"]
    iota = to_bf16(np.tile(np.arange(pre["QW"], dtype=np.float32)[None, :], (P, 1)))
    ident = np.eye(P, dtype=np.float32)
    x = np.asarray(x, np.float32)
    in_maps = []
    mb1w = np.ascontiguousarray(np.asarray(mb1, np.float32).reshape(2, P).T)
    mb2r = np.tile(np.asarray(mb2, np.float32).reshape(1, ncls), (P, 1))
    for c in range(ncores):
        xs = np.zeros((P, npc_pad), np.float32)
        xs[:, :npc] = x[c * npc:(c + 1) * npc, :].T
        m = dict(
            xT=to_bf16(xs),
            idxs=pre["cores"][c]["idxs"],
            colloc=to_bf16(pre["cores"][c]["colloc"]),
            wvals=to_bf16(pre["cores"][c]["wvals"]),
            batchloc=to_bf16(pre["cores"][c]["batchloc"]),
            growidx=pre["cores"][c]["growidx"],
            cntinv=pre["cntinv"],
            iota512=iota,
            ident128=ident,
            wi1=to_bf16(Wi1), wr1=to_bf16(Wr1),
            wi2=to_bf16(Wi2), wr2=to_bf16(Wr2),
            b1=np.asarray(b1, np.float32).reshape(P, 1),
            b2=np.asarray(b2, np.float32).reshape(P, 1),
            mw1=np.asarray(mW1, np.float32),
            mb1=mb1w,
            mw2=np.ascontiguousarray(
                np.asarray(mW2, np.float32).reshape(2, P, ncls).transpose(1, 0, 2)),
            mb2=mb2r,
        )
        in_maps.append(m)
    return in_maps


# ======================= entry point =======================
N_FULL = 100000
E_FULL = 640000
G_FULL = 512
NCLS_FULL = 2
NCORES = 8

_cache = {}


def kernel(x, edge_index, edge_attr, batch, Wi1, Wr1, b1, Wi2, Wr2, b2,
           mW1, mb1, mW2, mb2):
    install()
    x = np.asarray(x)
    edge_index = np.asarray(edge_index)
    edge_attr = np.asarray(edge_attr)
    batch = np.asarray(batch)
    n, f = x.shape
    g = G_FULL
    ncls = np.asarray(mW2).shape[1]

    pre = preprocess(edge_index, edge_attr, batch, n, NCORES, g)
    key = (n, g, ncls, pre["C"])
    if key not in _cache:
        nc = build_nc(pre, n, NCORES, g, ncls)
        _cache[key] = SpmdKernel(nc)
    k = _cache[key]
    in_maps = make_inputs(pre, x, Wi1, Wr1, b1, Wi2, Wr2, b2,
                          mW1, mb1, mW2, mb2, n, NCORES, g, ncls)
    ci, zz = k.put_inputs(in_maps)
    res = k.run_np(ci, zz)
    return np.ascontiguousarray(res[0]["out"].astype(np.float32))
